# revision 4
# baseline (speedup 1.0000x reference)
"""Trainium2 Bass kernel for BrainFunctionalConnectivityFeatureExtractionModule.

Math (per batch b, all f32):
    w    = relu(adj + adj_bias)                       (16,16)
    d    = 1/sqrt(sum(w, axis=1) + 1e-5)              (16,)
    lap  = I - d[:,None] * w * d[None,:]              (16,16)
    t1   = lap @ x[b]                                 (16,256)
    cp   = interleave(ones, t1)                       (16,512)
    h    = relu(brelu_bias + cp @ cheb_w)             (16,64)
    out  = h @ fc_w.T + fc_b                          (16,387)

Since the even interleaved lanes of cp are all-ones,
    cp @ cheb_w = t1 @ cheb_w[1::2] + sum(cheb_w[0::2], axis=0)
so the whole module collapses to three chained matmuls + relu:
    h   = relu(t1 @ W1 + bias_h),   W1 = cheb_w[1::2]  (256,64)
    out = h @ fc_w.T + fc_b

Device mapping: pure data parallel over 8 cores, B=8192 -> 1024 batches/core,
ROWS = 1024*16 = 16384 (b,e)-rows per core, processed in 512-row macro tiles
of 4 x 128-row sub-tiles (sub-tile = 8 full 16-node graphs).

The awkward step is t1 = lap @ x[b]: the E-contraction runs along the SBUF
partition axis of x.  We fold the 16x16 lap mix into a transposing matmul:
      t1T[c, n] = x_sub[:, c_chunk].T @ (I_8 (x) lap^T)      [n = row in sub]
which lands t1 TRANSPOSED ([C on partitions, rows free]) -- exactly the
layout the W1 matmul wants.  Stage 2: hT[64, 512] = sum_k W1_k^T @ t1T_k.
Stage 3: out[128, 388pad] = hT_slice.T @ fc_wT; +fc_b is fused into the
matmul via a constant all-ones 65th lhsT partition whose fc_w row is fc_b.

This kernel is HBM-bound, so all device I/O is 16-bit:
  - x is cast to bf16 on the HOST and DMA'd as bf16 (8.4 MB/core instead of
    16.8; the on-chip f32->bf16 cast stage of the earlier version is gone).
    The per-partition DMA runs are 512 B -- exactly at the DMA engines'
    full-rate descriptor threshold.
  - out is stored as bf16 (12.7 MB/core instead of 25.4) and upcast to f32
    on the host.  Rounding the output to bf16 adds ~1e-3 RMS relative error
    against a 2e-2 gate (measured total 3.3e-3).
PSUM->SBUF copy work is split across the three non-tensor engines so none
exceeds the ~1.8us/tile DMA floor: DVE does the fused 1024-elem t1 copy,
Act does relu+bias plus two out-copies, Pool does the other two out-copies.
The all-ones bias row of hT lives in 3 manually rotated persistent buffers,
memset once in the preamble instead of every tile.
All matmul inputs are bf16 (PSUM accumulation is f32): fp32/fp32r matmuls
hit a 2-4x slower datapath on trn2 and do not engage the PE clock-gate
release (measured: whole kernel stuck at K=4/8, 1.2 GHz).
"""

import numpy as np
from contextlib import ExitStack

B, E, C, H, OUT = 8192, 16, 256, 64, 387
NCORES = 8
ROWS = (B // NCORES) * E        # 16384 rows per core
NS = 4                          # sub-tiles per macro tile
TR = 128 * NS                   # 512 macro-tile rows
NT = ROWS // TR                 # 32 macro tiles per core
KC = C // 128                   # 2 contraction chunks of 128
OUTP = OUT + 1                  # fc matmul N padded even

_cache = {}


def _build_module(nt=NT):
    import concourse.tile as tile
    from concourse import bacc, mybir

    f32 = mybir.dt.float32
    bf16 = mybir.dt.bfloat16
    Relu = mybir.ActivationFunctionType.Relu

    nc = bacc.Bacc("TRN2", target_bir_lowering=False, debug=False,
                   num_devices=NCORES)

    rows = nt * TR
    x_d = nc.dram_tensor("x", (rows, C), bf16, kind="ExternalInput").ap()
    r_d = nc.dram_tensor("r", (128, 128), bf16, kind="ExternalInput").ap()
    w1_d = nc.dram_tensor("w1", (KC, 128, H), bf16, kind="ExternalInput").ap()
    bh_d = nc.dram_tensor("bh", (H, 1), f32, kind="ExternalInput").ap()
    fcw_d = nc.dram_tensor("fcw", (H + 1, OUTP), bf16, kind="ExternalInput").ap()
    o_d = nc.dram_tensor("o", (rows, OUT), bf16, kind="ExternalOutput").ap()

    with tile.TileContext(nc) as tc:
        with ExitStack() as ctx:
            consts = ctx.enter_context(tc.tile_pool(name="consts", bufs=1))
            xp = ctx.enter_context(tc.tile_pool(name="xp", bufs=4))
            t1sp = ctx.enter_context(tc.tile_pool(name="t1sp", bufs=3))
            op = ctx.enter_context(tc.tile_pool(name="op", bufs=3))
            t1pp = ctx.enter_context(tc.tile_pool(name="t1pp", bufs=2, space="PSUM"))
            hpp = ctx.enter_context(tc.tile_pool(name="hpp", bufs=2, space="PSUM"))
            opp = ctx.enter_context(tc.tile_pool(name="opp", bufs=2, space="PSUM"))

            r_sb = consts.tile([128, 128], bf16)
            nc.sync.dma_start(r_sb, r_d)
            w1_sb = consts.tile([128, KC, H], bf16)
            nc.sync.dma_start(w1_sb, w1_d.rearrange("k p h -> p k h"))
            bh_sb = consts.tile([H, 1], f32)
            nc.sync.dma_start(bh_sb, bh_d)
            fcw_sb = consts.tile([H + 1, OUTP], bf16)
            nc.sync.dma_start(fcw_sb, fcw_d)

            # hT buffers rotate manually so their all-ones fc_b row (the
            # 65th lhsT partition of stage 3) is written once, not per tile
            hbufs = []
            for i in range(3):
                hb = consts.tile([H + 1, TR], bf16, name=f"hb{i}")
                nc.gpsimd.memset(hb[H:H + 1, :], 1.0)
                hbufs.append(hb)

            # x: row l of macro t lives at sub-tile l//128, partition l%128
            xv = x_d.rearrange("(t s p) c -> t p s c", p=128, s=NS)
            # out: row l at partition l//4, slot l%4 -> 3KB contiguous runs
            ov = o_d.rearrange("(t p s) o -> t p s o", p=128, s=NS)

            for t in range(nt):
                x_sb = xp.tile([128, NS, C], bf16)
                nc.sync.dma_start(x_sb, xv[t])

                # stage 1: t1T[c, s*128+n] = x[:, s, c_chunk].T @ (I8 (x) lapT)
                t1_ps = t1pp.tile([128, KC, TR], f32)
                for k in range(KC):
                    for s in range(NS):
                        nc.tensor.matmul(
                            t1_ps[:, k, s * 128:(s + 1) * 128],
                            lhsT=x_sb[:, s, k * 128:(k + 1) * 128],
                            rhs=r_sb,
                        )
                t1_sb = t1sp.tile([128, KC, TR], bf16)
                nc.vector.tensor_copy(t1_sb, t1_ps)

                # stage 2: hT[h, n] = sum_k W1_k.T @ t1T_k
                h_ps = hpp.tile([H, TR], f32)
                for k in range(KC):
                    nc.tensor.matmul(
                        h_ps,
                        lhsT=w1_sb[:, k, :],
                        rhs=t1_sb[:, k, :],
                        start=(k == 0),
                        stop=(k == KC - 1),
                    )
                hT_sb = hbufs[t % 3]
                nc.vector.tensor_scalar(
                    hT_sb[0:H, :], h_ps, bh_sb, 0.0,
                    mybir.AluOpType.add, mybir.AluOpType.max)

                # stage 3: slot s covers rows l = 4p + s (hT cols s::4)
                o_sb = op.tile([128, NS, OUT], bf16)
                hT_v = hT_sb.rearrange("h (n s) -> h s n", s=NS)
                for s in range(NS):
                    o_ps = opp.tile([128, OUTP], f32)
                    nc.tensor.matmul(
                        o_ps,
                        lhsT=hT_v[:, s, :],
                        rhs=fcw_sb,
                    )
                    nc.scalar.copy(o_sb[:, s, :], o_ps[:, 0:OUT])
                nc.sync.dma_start(ov[t], o_sb)

    nc.finalize()
    return nc


def _host_prep(adj, adj_bias, cheb_w, brelu_bias, fc_w, fc_b):
    import ml_dtypes

    bf = ml_dtypes.bfloat16
    adj = np.asarray(adj, np.float32)
    w = np.maximum(adj + np.float32(adj_bias.reshape(())), 0.0)
    d = 1.0 / np.sqrt(w.sum(axis=1) + np.float32(1e-5))
    lap = np.eye(E, dtype=np.float32) - d[:, None] * w * d[None, :]

    # r = I_8 (x) lap^T : [p = b*16+j, n = b*16+i] -> lap[i, j]
    r = np.kron(np.eye(128 // E, dtype=np.float32), lap.T)

    cheb_w = np.asarray(cheb_w, np.float32)
    w1 = np.ascontiguousarray(cheb_w[1::2, :]).reshape(KC, 128, H)
    bias_h = (cheb_w[0::2, :].sum(axis=0)
              + np.asarray(brelu_bias, np.float32).reshape(H))
    fcw = np.zeros((H + 1, OUTP), np.float32)
    fcw[:H, :OUT] = np.asarray(fc_w, np.float32).T
    fcw[H, :OUT] = np.asarray(fc_b, np.float32)
    return {
        "r": r.astype(bf),
        "w1": np.ascontiguousarray(w1).astype(bf),
        "bh": bias_h.reshape(H, 1).astype(np.float32),
        "fcw": fcw.astype(bf),
    }


def _run(inputs, trace=False, nt=NT, **kw):
    import ml_dtypes
    from concourse import bass_utils

    if nt not in _cache:
        _cache[nt] = _build_module(nt=nt)
    nc = _cache[nt]

    x = np.asarray(inputs["x"], np.float32).astype(ml_dtypes.bfloat16)
    weights = _host_prep(inputs["adj"], inputs["adj_bias"], inputs["cheb_w"],
                         inputs["brelu_bias"], inputs["fc_w"], inputs["fc_b"])

    rows = nt * TR
    shards = x.reshape(NCORES, ROWS, C)[:, :rows]
    in_maps = [dict(weights, x=np.ascontiguousarray(shards[c]))
               for c in range(NCORES)]

    res = bass_utils.run_bass_kernel_spmd(
        nc, in_maps, core_ids=list(range(NCORES)), trace=trace, **kw)

    out = np.concatenate(
        [np.asarray(res.results[c]["o"], dtype=np.float32)
           .reshape(rows // E, E, OUT)
         for c in range(NCORES)], axis=0)
    return out, res


def kernel(**inputs) -> np.ndarray:
    out, _ = _run(inputs, trace=False)
    return out


# revision 16
# speedup vs baseline: 1.6525x; 1.6525x over previous
"""Trainium2 Bass kernel for BrainFunctionalConnectivityFeatureExtractionModule.

Math (per batch b, all f32):
    w    = relu(adj + adj_bias)                       (16,16)
    d    = 1/sqrt(sum(w, axis=1) + 1e-5)              (16,)
    lap  = I - d[:,None] * w * d[None,:]              (16,16)
    t1   = lap @ x[b]                                 (16,256)
    cp   = interleave(ones, t1)                       (16,512)
    h    = relu(brelu_bias + cp @ cheb_w)             (16,64)
    out  = h @ fc_w.T + fc_b                          (16,387)

Since the even interleaved lanes of cp are all-ones,
    cp @ cheb_w = t1 @ cheb_w[1::2] + sum(cheb_w[0::2], axis=0)
so the whole module collapses to three chained matmuls + relu:
    h   = relu(t1 @ W1 + bias_h),   W1 = cheb_w[1::2]  (256,64)
    out = h @ fc_w.T + fc_b

Device mapping: pure data parallel over 8 cores, B=8192 -> 1024 batches/core,
ROWS = 1024*16 = 16384 (b,e)-rows per core, processed in 512-row macro tiles
of 4 x 128-row sub-tiles (sub-tile = 8 full 16-node graphs).

The awkward step is t1 = lap @ x[b]: the E-contraction runs along the SBUF
partition axis of x.  We fold the 16x16 lap mix into a transposing matmul:
      t1T[c, n] = x_sub[:, c_chunk].T @ (I_8 (x) lap^T)      [n = row in sub]
which lands t1 TRANSPOSED ([C on partitions, rows free]) -- exactly the
layout the W1 matmul wants.  Stage 2: hT[64, 512] = sum_k W1_k^T @ t1T_k.
Stage 3: out[128, 388pad] = hT_slice.T @ fc_wT; +fc_b is fused into the
matmul via a constant all-ones 65th lhsT partition whose fc_w row is fc_b.

This kernel is HBM-bound, so all device I/O is 16-bit:
  - x is cast to bf16 on the HOST and DMA'd as bf16 (8.4 MB/core instead of
    16.8; the on-chip f32->bf16 cast stage of the earlier version is gone).
    The per-partition DMA runs are 512 B -- exactly at the DMA engines'
    full-rate descriptor threshold.
  - out is stored as bf16 (12.7 MB/core instead of 25.4) and upcast to f32
    on the host.  Rounding the output to bf16 adds ~1e-3 RMS relative error
    against a 2e-2 gate (measured total 3.3e-3).
Only DVE and Act can read PSUM on trn2 (the BIR verifier rejects GpSimd),
so the PSUM->SBUF copy work (t1 casts 1024 + relu-bias 512 + out-cast 1552
elems/partition/tile) is split: DVE takes the two t1 k-chunk casts and two
out-copies, Act takes relu+bias (activation with fused bias) and the other
two out-copies.  The loop is software-pipelined with a 2-tile lag (stage 1
of tile t, stage 2 of t-1, stage 3 of t-2) and the PE stream is ordered
s1k0(t), s3(t-2), s2(t-1), s1k1(t) so no PE instruction waits on a
cross-engine producer from the same iteration -- this keeps the PE stall-
free, which also keeps it at the 2.4 GHz p-state (stalls drop it to 1.2).
PSUM (8 banks): t1 k0 double- + k1 single-buffered (3), h (1), out slots
(4, one bank per slot so each stage-3 matmul reuses its slot's bank a full
tile later).  The all-ones fc_b row of hT lives in 3 manually rotated
persistent buffers, memset once in the preamble instead of every tile.
All matmul inputs are bf16 (PSUM accumulation is f32): fp32/fp32r matmuls
hit a 2-4x slower datapath on trn2 and do not engage the PE clock-gate
release (measured: whole kernel stuck at K=4/8, 1.2 GHz).
Measured via traced NT=8/16 runs: 2.82 us/tile steady state -> ~118 us
full scale, vs 195 us for the f32-I/O non-pipelined predecessor.
"""

import numpy as np
from contextlib import ExitStack

B, E, C, H, OUT = 8192, 16, 256, 64, 387
NCORES = 8
ROWS = (B // NCORES) * E        # 16384 rows per core
NS = 4                          # sub-tiles per macro tile
TR = 128 * NS                   # 512 macro-tile rows
NT = ROWS // TR                 # 32 macro tiles per core
KC = C // 128                   # 2 contraction chunks of 128
OUTP = OUT + 1                  # fc matmul N padded even

_cache = {}


def _build_module(nt=NT):
    import concourse.tile as tile
    from concourse import bacc, mybir

    f32 = mybir.dt.float32
    bf16 = mybir.dt.bfloat16
    Relu = mybir.ActivationFunctionType.Relu

    nc = bacc.Bacc("TRN2", target_bir_lowering=False, debug=False,
                   num_devices=NCORES)

    rows = nt * TR
    x_d = nc.dram_tensor("x", (rows, C), bf16, kind="ExternalInput").ap()
    r_d = nc.dram_tensor("r", (128, 128), bf16, kind="ExternalInput").ap()
    w1_d = nc.dram_tensor("w1", (KC, 128, H), bf16, kind="ExternalInput").ap()
    bh_d = nc.dram_tensor("bh", (H, 1), f32, kind="ExternalInput").ap()
    fcw_d = nc.dram_tensor("fcw", (H + 1, OUTP), bf16, kind="ExternalInput").ap()
    o_d = nc.dram_tensor("o", (rows, OUT), bf16, kind="ExternalOutput").ap()

    with tile.TileContext(nc) as tc:
        with ExitStack() as ctx:
            consts = ctx.enter_context(tc.tile_pool(name="consts", bufs=1))
            xp = ctx.enter_context(tc.tile_pool(name="xp", bufs=4))
            t1sp = ctx.enter_context(tc.tile_pool(name="t1sp", bufs=3))
            op = ctx.enter_context(tc.tile_pool(name="op", bufs=3))
            t1pp = ctx.enter_context(tc.tile_pool(name="t1pp", bufs=1, space="PSUM"))
            hpp = ctx.enter_context(tc.tile_pool(name="hpp", bufs=1, space="PSUM"))
            opp = ctx.enter_context(tc.tile_pool(name="opp", bufs=4, space="PSUM"))

            # weights go on the DVE/Act DGE queues so the SP queue's head
            # start feeding tile-0 x immediately (8 serial ~0.7us DMA issues
            # on SP cost ~6us of startup otherwise)
            r_sb = consts.tile([128, 128], bf16)
            nc.scalar.dma_start(r_sb, r_d)
            w1_sb = consts.tile([128, KC, H], bf16)
            nc.scalar.dma_start(w1_sb, w1_d.rearrange("k p h -> p k h"))
            bh_sb = consts.tile([H, 1], f32)
            nc.scalar.dma_start(bh_sb, bh_d)
            fcw_sb = consts.tile([H + 1, OUTP], bf16)
            nc.scalar.dma_start(fcw_sb, fcw_d)

            # hT buffers rotate manually so their all-ones fc_b row (the
            # 65th lhsT partition of stage 3) is written once, not per tile
            hbufs = []
            for i in range(3):
                hb = consts.tile([H + 1, TR], bf16, name=f"hb{i}")
                nc.gpsimd.memset(hb[H:H + 1, :], 1.0)
                hbufs.append(hb)

            # x: row l of macro t lives at sub-tile l//128, partition l%128
            xv = x_d.rearrange("(t s p) c -> t p s c", p=128, s=NS)
            # out: row l at partition l//4, slot l%4 -> 3KB contiguous runs
            ov = o_d.rearrange("(t p s) o -> t p s o", p=128, s=NS)

            # Software-pipelined over iterations `it`: stage 1 of tile it,
            # stage 2 of tile it-1, stage 3 of tile it-2.  The scheduler
            # preserves per-engine program order, so within an iteration the
            # PE stream is ordered  s1k0(it), s3(it-2), s2(it-1), s1k1(it)
            # such that no PE instruction waits on a cross-engine producer
            # scheduled in the SAME iteration: s3's inputs (hT, opp banks)
            # and s2's inputs (t1 casts) are all a full period old.  t1 PSUM
            # is split per-k (k0 double-, k1 single-buffered; k1 is produced
            # at the end of the PE stream and cast early the next iteration)
            # to free a 4th PSUM bank for opp, so each stage-3 matmul reuses
            # the bank its own slot used one full tile earlier.
            t1ps_q, casted = {}, {}
            for it in range(nt + 2):
                j, g = it - 1, it - 2

                if it < nt:
                    # stage 1 (k=0 half): t1T[c, n] = x[:,s,c0].T @ (I8 (x) lapT)
                    x_sb = xp.tile([128, NS, C], bf16)
                    nc.sync.dma_start(x_sb, xv[it])
                    t1k0_ps = t1pp.tile([128, TR], f32, name="t1k0", bufs=2)
                    for s in range(NS):
                        nc.tensor.matmul(
                            t1k0_ps[:, s * 128:(s + 1) * 128],
                            lhsT=x_sb[:, s, 0:128],
                            rhs=r_sb,
                        )
                    t1ps_q[it] = (x_sb, t1k0_ps)

                if 0 <= j < nt:
                    # casts of tile j's t1 (written by the tail of the
                    # previous PE iteration) run first on DVE
                    t1_sb = t1sp.tile([128, KC, TR], bf16)
                    xj_sb, t1k0j, t1k1j = casted.pop(j)
                    nc.vector.tensor_copy(t1_sb[:, 0, :], t1k0j)
                    nc.vector.tensor_copy(t1_sb[:, 1, :], t1k1j)

                if 0 <= g < nt:
                    # stage 3: slot s covers rows l = 4p + s (hT cols s::4).
                    # opp bufs=4 gives each slot its own PSUM bank, so every
                    # stage-3 matmul reuses the bank its slot used one full
                    # tile earlier (copies never gate the PE mid-tile).
                    o_sb = op.tile([128, NS, OUT], bf16)
                    hT_v = hbufs[g % 3].rearrange("h (n s) -> h s n", s=NS)
                    for s in range(NS):
                        o_ps = opp.tile([128, OUTP], f32)
                        nc.tensor.matmul(
                            o_ps,
                            lhsT=hT_v[:, s, :],
                            rhs=fcw_sb,
                        )
                        if s < 2:
                            nc.scalar.copy(o_sb[:, s, :], o_ps[:, 0:OUT])
                        else:
                            nc.vector.tensor_copy(o_sb[:, s, :], o_ps[:, 0:OUT])
                    nc.sync.dma_start(ov[g], o_sb)

                if 0 <= j < nt:
                    # stage 2: hT[h, n] = relu(bh + sum_k W1_k.T @ t1T_k)
                    h_ps = hpp.tile([H, TR], f32)
                    for k in range(KC):
                        nc.tensor.matmul(
                            h_ps,
                            lhsT=w1_sb[:, k, :],
                            rhs=t1_sb[:, k, :],
                            start=(k == 0),
                            stop=(k == KC - 1),
                        )
                    nc.scalar.activation(hbufs[j % 3][0:H, :], h_ps, Relu,
                                         bias=bh_sb)

                if it < nt:
                    # stage 1 (k=1 half) last: its cast runs first thing
                    # next iteration, so the single PSUM buffer is safe
                    x_sb, t1k0_ps = t1ps_q.pop(it)
                    t1k1_ps = t1pp.tile([128, TR], f32, name="t1k1", bufs=1)
                    for s in range(NS):
                        nc.tensor.matmul(
                            t1k1_ps[:, s * 128:(s + 1) * 128],
                            lhsT=x_sb[:, s, 128:256],
                            rhs=r_sb,
                        )
                    casted[it] = (x_sb, t1k0_ps, t1k1_ps)

    nc.finalize()
    return nc


def _host_prep(adj, adj_bias, cheb_w, brelu_bias, fc_w, fc_b):
    import ml_dtypes

    bf = ml_dtypes.bfloat16
    adj = np.asarray(adj, np.float32)
    w = np.maximum(adj + np.float32(adj_bias.reshape(())), 0.0)
    d = 1.0 / np.sqrt(w.sum(axis=1) + np.float32(1e-5))
    lap = np.eye(E, dtype=np.float32) - d[:, None] * w * d[None, :]

    # r = I_8 (x) lap^T : [p = b*16+j, n = b*16+i] -> lap[i, j]
    r = np.kron(np.eye(128 // E, dtype=np.float32), lap.T)

    cheb_w = np.asarray(cheb_w, np.float32)
    w1 = np.ascontiguousarray(cheb_w[1::2, :]).reshape(KC, 128, H)
    bias_h = (cheb_w[0::2, :].sum(axis=0)
              + np.asarray(brelu_bias, np.float32).reshape(H))
    fcw = np.zeros((H + 1, OUTP), np.float32)
    fcw[:H, :OUT] = np.asarray(fc_w, np.float32).T
    fcw[H, :OUT] = np.asarray(fc_b, np.float32)
    return {
        "r": r.astype(bf),
        "w1": np.ascontiguousarray(w1).astype(bf),
        "bh": bias_h.reshape(H, 1).astype(np.float32),
        "fcw": fcw.astype(bf),
    }


def _run(inputs, trace=False, nt=NT, **kw):
    import ml_dtypes
    from concourse import bass_utils

    if nt not in _cache:
        _cache[nt] = _build_module(nt=nt)
    nc = _cache[nt]

    x = np.asarray(inputs["x"], np.float32).astype(ml_dtypes.bfloat16)
    weights = _host_prep(inputs["adj"], inputs["adj_bias"], inputs["cheb_w"],
                         inputs["brelu_bias"], inputs["fc_w"], inputs["fc_b"])

    rows = nt * TR
    shards = x.reshape(NCORES, ROWS, C)[:, :rows]
    in_maps = [dict(weights, x=np.ascontiguousarray(shards[c]))
               for c in range(NCORES)]

    res = bass_utils.run_bass_kernel_spmd(
        nc, in_maps, core_ids=list(range(NCORES)), trace=trace, **kw)

    out = np.concatenate(
        [np.asarray(res.results[c]["o"], dtype=np.float32)
           .reshape(rows // E, E, OUT)
         for c in range(NCORES)], axis=0)
    return out, res


def kernel(**inputs) -> np.ndarray:
    out, _ = _run(inputs, trace=False)
    return out
